# revision 29
# baseline (speedup 1.0000x reference)
"""AttentionXL Trainium2 kernel: 8-core hybrid batch x head parallel, v3.

Self-contained: hardcodes shapes from the problem spec.
  inputs:       (1024, 4, 1024) f32   cur_seq, bs, d
  full_input:   (2048, 4, 1024) f32   full_seq, bs, d
  pos_embedding:(2048, 1024)    f32
  u, v:         (16, 64)        f32   H, D
  Wkv (1024, 2*1024), Wq/Wr/Wo (1024, 1024), biases zero, mask all-False.

Sharding: core c handles batch b_c = c//2 and head-group hg_c = c%2
(8 heads = 4 dc-blocks of 128 = 2 heads each). Each core loads ONLY its
batch's x_cur/x_full (replicated pos), so input DMA is 10MB/core instead
of 28MB; partial outputs are [d, cs] per core, host sums the 2 cores of
each batch and adds bo.

Per core:
  qTu/qTv = Wq_hg^T x_cur^T (+u/+v)   [128, 4*cs]  (block-major cols)
  kT      = Wk_hg^T x_full^T          [128, 4*fs]
  rT      = Wr_hg^T pos^T             [128, 4*fs]
  V       = x_full^T^T Wv_blk per 128-token tile -> vte [128 j, 65] per
            (jt, h) with ones column (AV stationary, no PE transposes)
  Per head h (blk = h//2, 64-row slice):
    EB  = exp(((q+v)^T r)/8) in [i, m] layout; Act exp-evict
          -> DRAM pitch fs+1 (col 0 = exp(0) = 1)   [rel-shift pitch trick]
    EBs^T <- DMA-transpose read of the shifted view (exact reference
          rel_shift flat-reinterpret incl. wrap)
    EC  = exp((K^T(q+u))/8) per j-tile  [j, i]  (Act exp-evict from PSUM)
    E   = EC * EBs^T  (DVE bf16 multiply; exp(a+b) = exp(a)exp(b) — no
          identity-add matmul on the PE)
    O^T[65] = [V|1]^T E  (AV matmul; row 64 = softmax denominators)
    attn_vec = O^T[0:64] / Z
  Y^T_partial = Wo_hg^T attn_vec  (accumulate 4 dc-blocks in PSUM) -> DRAM
"""

import os
from contextlib import ExitStack

import numpy as np
import ml_dtypes

import concourse.bass as bass
import concourse.bacc as bacc_mod
import concourse.mybir as mybir
import concourse.tile as tile

BF16 = mybir.dt.bfloat16
F32 = mybir.dt.float32
NPBF16 = ml_dtypes.bfloat16

# Problem dims (full size)
CS, FS, BS, D_MODEL = 1024, 2048, 4, 1024
H, HD = 16, 64
N_CORES = 8
NBG = 4                     # batch groups
NHG = 2                     # head groups
HPC = H // NHG              # heads per core = 8
NB = HPC * HD // 128        # dc-blocks per core = 4
DCB = 128                   # dims per block
DC = NB * DCB               # per-core model slice = 512


def build_core_kernel(cs=CS, fs=FS, d=D_MODEL, hpc=HPC, hd=HD, loop=1):
    nb = NB
    nk = d // 128           # contraction chunks for projections
    NJ = fs // 128          # j tiles
    NI = cs // 128          # i tiles
    NIC = cs // 512         # 512-wide i chunks
    NT = fs // 128          # kv token tiles
    scale = 1.0 / (hd ** 0.5)

    nc = bacc_mod.Bacc(None, target_bir_lowering=False, debug=False)

    xcurT = nc.dram_tensor("xcurT", [d, cs], BF16, kind="ExternalInput")
    xfullT = nc.dram_tensor("xfullT", [d, fs], BF16, kind="ExternalInput")
    posT = nc.dram_tensor("posT", [d, fs], BF16, kind="ExternalInput")
    wq_d = nc.dram_tensor("wq", [d, DC], BF16, kind="ExternalInput")
    wk_d = nc.dram_tensor("wk", [d, DC], BF16, kind="ExternalInput")
    wv_d = nc.dram_tensor("wv", [d, DC], BF16, kind="ExternalInput")
    wr_d = nc.dram_tensor("wr", [d, DC], BF16, kind="ExternalInput")
    wo_d = nc.dram_tensor("wo", [DC, d], BF16, kind="ExternalInput")
    u_d = nc.dram_tensor("u", [DC, 1], F32, kind="ExternalInput")
    v_d = nc.dram_tensor("v", [DC, 1], F32, kind="ExternalInput")
    y_d = nc.dram_tensor("y", [d, cs], BF16, kind="ExternalOutput")

    # DRAM scratch for the rel-shift pitch trick (multiplicative domain):
    # [cs rows, fs+1 cols]; col 0 stays exp(0) = 1.
    p2 = [nc.dram_tensor(f"p2_{i}", [cs * (fs + 1)], BF16) for i in range(8)]

    Exp = mybir.ActivationFunctionType.Exp

    with tile.TileContext(nc) as tc, ExitStack() as ctx:
        const = ctx.enter_context(tc.tile_pool(name="const", bufs=1))
        persist = ctx.enter_context(tc.tile_pool(name="persist", bufs=1))
        wts = ctx.enter_context(tc.tile_pool(name="wts", bufs=2))
        xs = ctx.enter_context(tc.tile_pool(name="xs", bufs=11))
        xfr = ctx.enter_context(tc.tile_pool(name="xfr", bufs=8))
        ebp = ctx.enter_context(tc.tile_pool(name="ebp", bufs=3))
        bdst = ctx.enter_context(tc.tile_pool(name="bdst", bufs=8))
        ecp = ctx.enter_context(tc.tile_pool(name="ecp", bufs=4))
        ea = ctx.enter_context(tc.tile_pool(name="ea", bufs=6))
        onrm = ctx.enter_context(tc.tile_pool(name="onrm", bufs=2))
        yout = ctx.enter_context(tc.tile_pool(name="yout", bufs=2))
        psAB = ctx.enter_context(tc.tile_pool(name="psAB", bufs=3, space="PSUM"))
        psS = ctx.enter_context(tc.tile_pool(name="psS", bufs=2, space="PSUM"))

        # ---- small constants ----
        u_sb = const.tile([128, nb], F32)   # col blk = u slice of block blk
        v_sb = const.tile([128, nb], F32)
        usrc = bass.AP(tensor=u_d, offset=0, ap=[[1, 128], [128, nb]])
        vsrc = bass.AP(tensor=v_d, offset=0, ap=[[1, 128], [128, nb]])
        nc.sync.dma_start(out=u_sb[:], in_=usrc)
        nc.sync.dma_start(out=v_sb[:], in_=vsrc)

        wo = const.tile([128, nb * d], BF16)  # block blk at cols [blk*d,)
        wosrc = bass.AP(tensor=wo_d, offset=0,
                        ap=[[d, 128], [128 * d, nb], [1, d]])
        nc.sync.dma_start(out=wo[:], in_=wosrc)

        # set col 0 of each p2 buffer to 1.0 (exp of the zero pad)
        zc = cs // 128
        onecol = const.tile([128, zc], BF16)
        nc.vector.memset(onecol[:], 1.0)
        for pb in p2:
            dst = bass.AP(tensor=pb, offset=0,
                          ap=[[fs + 1, 128], [(fs + 1) * 128, zc]])
            nc.sync.dma_start(out=dst, in_=onecol[:])

        # ---- persistent activations ----
        qTu = persist.tile([128, nb * cs], BF16)   # (q+u)^T, block-major
        qTv = persist.tile([128, nb * cs], BF16)
        kT = persist.tile([128, nb * fs], BF16)
        rT = persist.tile([128, nb * fs], BF16)
        # vte: per token tile t, cols [t*520 + h*65, +65) = [V_h | 1]
        vte = persist.tile([128, NT * hpc * 65], BF16)
        ofin = persist.tile([128, nb * cs], BF16)  # attn_vec^T, block-major

        def load_w(dram, nm):  # [d, DC] -> SBUF [128, nk*nb*128]
            # chunk kk, block blk at cols (kk*nb + blk)*128
            t = wts.tile([128, nk * DC], BF16, name=nm, tag="w")
            src = bass.AP(
                tensor=dram, offset=0,
                ap=[[DC, 128], [128 * DC, nk], [1, DC]],
            )
            nc.sync.dma_start(out=t[:], in_=src)
            return t

        def _phases():
            # ones columns of vte
            vte3 = vte[:].rearrange("p (t c) -> p t c", c=65)
            nc.vector.memset(vte3[:, :, 64:65], 1.0)

            CW = 1024

            # ---- q projection ----
            wq = load_w(wq_d, "wq_sb")
            for c0 in range(0, cs, CW):
                cw = min(CW, cs - c0)
                xts = []
                for kk in range(nk):
                    xt = xs.tile([128, cw], BF16)
                    nc.sync.dma_start(
                        out=xt[:], in_=xcurT[kk * 128:(kk + 1) * 128, c0:c0 + cw])
                    xts.append(xt)
                for blk in range(nb):
                    for s0 in range(0, cw, 512):
                        ps = psAB.tile([128, 512], F32, name="psq", tag="ab")
                        for kk in range(nk):
                            nc.tensor.matmul(
                                ps[:], wq[:, (kk * nb + blk) * 128:
                                          (kk * nb + blk + 1) * 128],
                                xts[kk][:, s0:s0 + 512],
                                start=(kk == 0), stop=(kk == nk - 1))
                        sl = (slice(None),
                              slice(blk * cs + c0 + s0, blk * cs + c0 + s0 + 512))
                        nc.vector.tensor_scalar_add(
                            qTu[sl], ps[:], u_sb[:, blk:blk + 1])
                        nc.vector.tensor_scalar_add(
                            qTv[sl], ps[:], v_sb[:, blk:blk + 1])

            # ---- r projection (generator; tail woven with bd head 0) ----
            wr = load_w(wr_d, "wr_sb")

            def gen_r():
                for c0 in range(0, fs, CW):
                    cw = min(CW, fs - c0)
                    xts = []
                    for kk in range(nk):
                        xt = xs.tile([128, cw], BF16)
                        nc.sync.dma_start(
                            out=xt[:],
                            in_=posT[kk * 128:(kk + 1) * 128, c0:c0 + cw])
                        xts.append(xt)
                    for blk in range(nb):
                        for s0 in range(0, cw, 512):
                            ps = psKV.tile([128, 512], F32, name="psr",
                                           tag="kv")
                            for kk in range(nk):
                                nc.tensor.matmul(
                                    ps[:], wr[:, (kk * nb + blk) * 128:
                                              (kk * nb + blk + 1) * 128],
                                    xts[kk][:, s0:s0 + 512],
                                    start=(kk == 0), stop=(kk == nk - 1))
                            nc.vector.tensor_copy(
                                rT[:, blk * fs + c0 + s0:
                                   blk * fs + c0 + s0 + 512],
                                ps[:])
                        yield

            # resident x_full tiles (kv projection interleaves with BD below)
            xfs = []
            for kk in range(nk):
                xt = xfr.tile([128, fs], BF16, name="xf", tag="xf")
                nc.sync.dma_start(
                    out=xt[:], in_=xfullT[kk * 128:(kk + 1) * 128, :])
                xfs.append(xt)

            wk = load_w(wk_d, "wk_sb")
            wv = load_w(wv_d, "wv_sb")

            def gen_kv(blk):
                for s0 in range(0, fs, 512):
                    psk = psAB.tile([128, 512], F32, name="psk", tag="ab")
                    for kk in range(nk):
                        nc.tensor.matmul(
                            psk[:], wk[:, (kk * nb + blk) * 128:
                                       (kk * nb + blk + 1) * 128],
                            xfs[kk][:, s0:s0 + 512],
                            start=(kk == 0), stop=(kk == nk - 1))
                    nc.vector.tensor_copy(
                        kT[:, blk * fs + s0:blk * fs + s0 + 512], psk[:])
                    yield
                # v re-oriented: out [tok 128, 128 dc] per token tile
                for g0 in range(0, fs, 512):
                    for s0 in range(g0, g0 + 512, 128):
                        t = s0 // 128
                        psv = psAB.tile([128, 128], F32, name="psv", tag="ab")
                        for kk in range(nk):
                            nc.tensor.matmul(
                                psv[:], xfs[kk][:, s0:s0 + 128],
                                wv[:, (kk * nb + blk) * 128:
                                   (kk * nb + blk + 1) * 128],
                                start=(kk == 0), stop=(kk == nk - 1))
                        c = (t * hpc + blk * 2) * 65
                        nc.vector.tensor_copy(vte[:, c:c + 64], psv[:, 0:hd])
                        nc.vector.tensor_copy(vte[:, c + 65:c + 129],
                                              psv[:, hd:2 * hd])
                    yield

            def gen_bd(h):
                blk = h // 2
                hs = slice((h % 2) * hd, (h % 2) * hd + hd)
                pb = p2[h]
                qb = blk * cs
                kb = blk * fs
                # EB = exp(BD/8) in [i, m] layout -> DRAM pitch fs+1
                for it in range(NI):
                    eb = ebp.tile([128, fs], BF16)
                    for half in range(2):
                        psb = psAB.tile([128, 1024], F32, name="psb", tag="ab")
                        for mc in range(2):
                            m0 = half * 1024 + mc * 512
                            nc.tensor.matmul(
                                psb[:, mc * 512:(mc + 1) * 512],
                                qTv[hs, qb + it * 128:qb + (it + 1) * 128],
                                rT[hs, kb + m0:kb + m0 + 512],
                                start=True, stop=True)
                        nc.scalar.activation(
                            eb[:, half * 1024:(half + 1) * 1024], psb[:],
                            Exp, scale=scale)
                        if half == 1:
                            dst = bass.AP(
                                tensor=pb,
                                offset=(it * 128) * (fs + 1) + 1,
                                ap=[[fs + 1, 128], [1, fs]])
                            nc.sync.dma_start(out=dst, in_=eb[:])
                        yield

            def gen_att(h, pso_pool=None):
                pool = pso_pool if pso_pool is not None else psS
                blk = h // 2
                hs = slice((h % 2) * hd, (h % 2) * hd + hd)
                pb = p2[h]
                qb = blk * cs
                kb = blk * fs
                PF = 4   # tr-read prefetch depth
                LAG = 4  # AV emission lag (hide PE in-order stalls)

                def trread(jt):
                    t = bdst.tile([128, cs], BF16)
                    srcap = bass.AP(tensor=pb, offset=cs + jt * 128,
                                    ap=[[fs, cs], [1, 128]])
                    nc.sync.dma_start(out=t[:], in_=srcap, transpose=True)
                    return t

                bds = {jt: trread(jt) for jt in range(PF)}
                pso = [pool.tile([65, 512], F32, name=f"pso{ic}",
                                 tag="s" if pso_pool is None else "ab")
                       for ic in range(NIC)]
                ets = {}
                for jt in range(NJ + LAG):
                    if jt < NJ:
                        if jt + PF < NJ:
                            bds[jt + PF] = trread(jt + PF)
                        psa = psAB.tile([128, 1024], F32, name="psa", tag="ab")
                        for ic in range(2):
                            nc.tensor.matmul(
                                psa[:, ic * 512:(ic + 1) * 512],
                                kT[hs, kb + jt * 128:kb + (jt + 1) * 128],
                                qTu[hs, qb + ic * 512:qb + (ic + 1) * 512],
                                start=True, stop=True)
                        ec = ecp.tile([128, cs], BF16)
                        nc.scalar.activation(ec[:], psa[:], Exp, scale=scale)
                        et = ea.tile([128, cs], BF16)
                        nc.vector.tensor_mul(et[:], ec[:], bds.pop(jt)[:])
                        ets[jt] = et
                    if jt >= LAG:
                        j2 = jt - LAG
                        tv = (j2 * hpc + h) * 65
                        for ic in range(NIC):
                            nc.tensor.matmul(
                                pso[ic][:], vte[:, tv:tv + 65],
                                ets[j2][:, ic * 512:(ic + 1) * 512],
                                start=(j2 == 0), stop=(j2 == NJ - 1))
                        del ets[j2]
                    yield

                # softmax normalize (row 64 = denominators)
                for ic in range(NIC):
                    ov = onrm.tile([65, 512], F32)
                    nc.vector.tensor_copy(ov[:], pso[ic][:])
                    rc = onrm.tile([1, 512], F32)
                    nc.vector.reciprocal(rc[:], ov[hd:hd + 1, :])
                    rb = onrm.tile([hd, 512], F32)
                    nc.gpsimd.partition_broadcast(rb[:], rc[:])
                    nc.vector.tensor_mul(
                        ofin[hs, qb + ic * 512:qb + (ic + 1) * 512],
                        ov[0:hd, :], rb[:])
                yield

            def gen_wo():
                wo = wts.tile([128, nb * d], BF16, name="wo_sb", tag="w")
                wosrc = bass.AP(tensor=wo_d, offset=0,
                                ap=[[d, 128], [128 * d, nb], [1, d]])
                nc.sync.dma_start(out=wo[:], in_=wosrc)
                for oc in range(d // 128):
                    yt = yout.tile([128, cs], BF16)
                    for ic in range(NIC):
                        psy = psS.tile([128, 512], F32, name="psy", tag="s")
                        for blk in range(nb):
                            nc.tensor.matmul(
                                psy[:],
                                wo[:, blk * d + oc * 128:blk * d + (oc + 1) * 128],
                                ofin[:, blk * cs + ic * 512:
                                     blk * cs + (ic + 1) * 512],
                                start=(blk == 0), stop=(blk == nb - 1))
                        nc.vector.tensor_copy(yt[:, ic * 512:(ic + 1) * 512],
                                              psy[:])
                    nc.sync.dma_start(
                        out=y_d[oc * 128:(oc + 1) * 128, :], in_=yt[:])
                    yield

            def weave(specs):
                # specs: list of (generator, units_per_round); round-robin
                # until all exhausted.
                specs = [[g, w] for g, w in specs]
                while specs:
                    done = []
                    for s in specs:
                        g, w = s
                        for _ in range(w):
                            try:
                                next(g)
                            except StopIteration:
                                done.append(s)
                                break
                    for s in done:
                        specs.remove(s)

            # stage blk: BD of heads 2blk/2blk+1 (Act-heavy) woven with the
            # kv projection of blk (PE-heavy) and the attention of the heads
            # whose kT/vte became ready in the previous stage.
            gr = gen_r()
            for _ in range(nb + 1):   # c0=0 all blks + c0=1 blk0
                next(gr)
            weave([(gr, 1), (gen_bd(0), 6)])
            for blk in range(nb):
                specs = [(gen_bd(2 * blk + 1), 2), (gen_kv(blk), 2)]
                if blk >= 1:
                    specs += [(gen_bd(2 * blk), 2), (gen_att(2 * blk - 2), 3)]
                weave(specs)
                if blk >= 1:
                    weave([(gen_att(2 * blk - 1), 1)])
            weave([(gen_att(hpc - 2), 1)])
            weave([(gen_att(hpc - 1), 1)])
            weave([(gen_wo(), 1)])

        for _rep in range(loop):
            _phases()

    nc.compile()
    return nc


_NC_CACHE = {}


def _get_nc(dims):
    if dims not in _NC_CACHE:
        _NC_CACHE[dims] = build_core_kernel(*dims)
    return _NC_CACHE[dims]


def make_in_maps(inputs, pos_embedding, full_input, u, v, Wkv, Wq, Wr, Wo,
                 cs=CS, fs=FS, bs=BS, d=D_MODEL, n_cores=N_CORES):
    xcur = np.asarray(inputs, np.float32)          # (cs, bs, d)
    xfull = np.asarray(full_input, np.float32)     # (fs, bs, d)
    posT = np.ascontiguousarray(
        np.asarray(pos_embedding, np.float32).T).astype(NPBF16)
    Wkv = np.asarray(Wkv, np.float32)
    Wq = np.asarray(Wq, np.float32)
    Wr = np.asarray(Wr, np.float32)
    Wo = np.asarray(Wo, np.float32)
    u = np.asarray(u, np.float32)
    v = np.asarray(v, np.float32)

    # per-batch transposed inputs
    xcurT_b = [np.ascontiguousarray(xcur[:, b, :].T).astype(NPBF16)
               for b in range(bs)]
    xfullT_b = [np.ascontiguousarray(xfull[:, b, :].T).astype(NPBF16)
                for b in range(bs)]

    in_maps = []
    for c in range(n_cores):
        bc, hg = c // NHG, c % NHG
        cols = slice(hg * DC, (hg + 1) * DC)
        in_maps.append({
            "xcurT": xcurT_b[bc],
            "xfullT": xfullT_b[bc],
            "posT": posT,
            "wq": np.ascontiguousarray(Wq[:, cols]).astype(NPBF16),
            "wk": np.ascontiguousarray(Wkv[:, hg * DC:(hg + 1) * DC]).astype(NPBF16),
            "wv": np.ascontiguousarray(
                Wkv[:, d + hg * DC:d + (hg + 1) * DC]).astype(NPBF16),
            "wr": np.ascontiguousarray(Wr[:, cols]).astype(NPBF16),
            "wo": np.ascontiguousarray(Wo[hg * DC:(hg + 1) * DC, :]).astype(NPBF16),
            "u": np.ascontiguousarray(
                u[hg * HPC:(hg + 1) * HPC].reshape(DC, 1)).astype(np.float32),
            "v": np.ascontiguousarray(
                v[hg * HPC:(hg + 1) * HPC].reshape(DC, 1)).astype(np.float32),
        })
    return in_maps


def combine_outputs(results, bo, cs=CS, bs=BS, d=D_MODEL):
    out = np.zeros((cs, bs, d), np.float32)
    for c, r in enumerate(results):
        bc = c // NHG
        out[:, bc, :] += np.asarray(r["y"], np.float32).T
    return (out + np.asarray(bo, np.float32)[None, None, :]).astype(np.float32)


def _build_runner(nc, n_cores, reps=1):
    """jit-compiled sharded executor for the prebuilt bass module (cached)."""
    import jax
    from jax.sharding import Mesh, PartitionSpec, NamedSharding
    from jax.experimental.shard_map import shard_map
    from concourse import bass2jax

    bass2jax.install_neuronx_cc_hook()
    partition_name = (nc.partition_id_tensor.name
                      if nc.partition_id_tensor else None)
    in_names, out_names, out_avals, zero_outs = [], [], [], []
    for alloc in nc.m.functions[0].allocations:
        if not isinstance(alloc, mybir.MemoryLocationSet):
            continue
        name = alloc.memorylocations[0].name
        if alloc.kind == "ExternalInput":
            if name != partition_name:
                in_names.append(name)
        elif alloc.kind == "ExternalOutput":
            shape = tuple(alloc.tensor_shape)
            dtype = mybir.dt.np(alloc.dtype)
            out_names.append(name)
            out_avals.append(jax.core.ShapedArray(shape, dtype))
            zero_outs.append(np.zeros(shape, dtype))
    n_params = len(in_names)
    all_names = list(in_names) + list(out_names)
    if partition_name is not None:
        all_names.append(partition_name)

    def _body(*args):
        outs = None
        for _ in range(reps):
            operands = list(args)
            if partition_name is not None:
                operands.append(bass2jax.partition_id_tensor())
            outs = bass2jax._bass_exec_p.bind(
                *operands,
                out_avals=tuple(out_avals),
                in_names=tuple(all_names),
                out_names=tuple(out_names),
                lowering_input_output_aliases=(),
                sim_require_finite=True,
                sim_require_nnan=True,
                nc=nc,
            )
        return tuple(outs)

    devices = jax.devices()[:n_cores]
    mesh = Mesh(np.asarray(devices), ("core",))
    n_outs = len(out_avals)
    fn = jax.jit(
        shard_map(_body, mesh=mesh,
                  in_specs=(PartitionSpec("core"),) * (n_params + n_outs),
                  out_specs=(PartitionSpec("core"),) * n_outs,
                  check_rep=False),
        keep_unused=True)
    sharding = NamedSharding(mesh, PartitionSpec("core"))

    def runner(in_maps):
        import jax as _jax
        per_core = [[np.asarray(m[name]) for name in in_names] for m in in_maps]
        args = [np.concatenate([per_core[c][i] for c in range(n_cores)], axis=0)
                for i in range(n_params)]
        args += [np.zeros((n_cores * z.shape[0], *z.shape[1:]), z.dtype)
                 for z in zero_outs]
        placed = [_jax.device_put(a, sharding) for a in args]
        out = fn(*placed)
        _jax.block_until_ready(out)
        return [
            {name: np.asarray(out[i]).reshape(n_cores, *out_avals[i].shape)[c]
             for i, name in enumerate(out_names)}
            for c in range(n_cores)
        ]

    return runner


_RUNNER_CACHE = {}


def _get_runner(dims):
    if dims not in _RUNNER_CACHE:
        nc = _get_nc(dims)
        _RUNNER_CACHE[dims] = _build_runner(nc, N_CORES)
    return _RUNNER_CACHE[dims]


def kernel(**inputs):
    dims = (CS, FS, D_MODEL, HPC, HD)
    runner = _get_runner(dims)
    in_maps = make_in_maps(
        inputs["inputs"], inputs["pos_embedding"], inputs["full_input"],
        inputs["u"], inputs["v"], inputs["Wkv"], inputs["Wq"], inputs["Wr"],
        inputs["Wo"])
    results = runner(in_maps)
    return combine_outputs(results, inputs["bo"])
